# revision 22
# baseline (speedup 1.0000x reference)
"""Trainium2 Bass kernel for nn_DPT_52845277610695 (topk_masking).

Self-contained: accepts FULL unsharded inputs, shards (b,m) pairs across
8 NeuronCores (2 pairs per core), runs one SPMD Bass kernel, and
assembles the final outputs on host.

Math: the reference's inner value_and_grad only touches `noise` at node
rows i in [T, T+t] = [32, 36], and every candidate score decomposes as
    opt_logp[b,m,k,j] = const + lp_graph[b,m] + opc[k,j]
                        - 0.5*A2S - CRS + 49.5*SQS + lp_external
where CRS[k,j] = sum_i v_i(k,j) . noise_row and SQS[k,j] = sum_i |noise_row|^2
are the only O(noise) quantities (computed on device via PE matvecs over
transposed noise tiles + ACT squares), and lp_external needs only a
12-row logsumexp over the vocab per (b,m) (computed on device).
The gradient is nonzero for at most ~14 rows per selected candidate and
is assembled on host for the 16 selected candidates only.

Device I/O is packed into single large-descriptor DMAs (HW-DGE
descriptor generation is the wall-clock bottleneck otherwise).
"""
import math
from contextlib import ExitStack

import numpy as np

PI = 3.1415926
EPS = 1e-8
B, M, L, K, E, V = 2, 8, 32, 4, 64, 1024
L2, K2 = 2 * L, 2 * K
BETASQ = 1.0
BETASQ2 = 100.0
INNER_LR = 1e-3
NCORES = 8
S_PER_CORE = (B * M) // NCORES  # 2

ASC = math.sqrt((BETASQ2 - BETASQ) / 2.0)   # activation square scale

# ---- packed smalls-A layout (f32 columns of a (128, SWA) tile) ----
_C_VST = 0            # (128, 7): cols 0..3 = square-bias cols -v/(2*ASC)
#                       col 4 = onesA (top half), 5 = onesB (bottom), 6 = ones
_C_V36 = 7            # (128, 64) per s: zero-padded lhsT (NEGATED V36T)
SWA = 7 + 64 * S_PER_CORE
# ---- packed smalls-B layout ((64, SWB) tile) ----
_C_EMB = 0                              # (64, 512) f32 = (64,1024) bf16
_C_GSEL = 512                           # (64, 6S) f32 = (64,12S) bf16
SWB = 512 + 6 * S_PER_CORE

# ---- packed output layout: one row (1, OW) ----
# pch: per-s PE chain = 49.5*SQS - CRS + CB (all blocks + masked 36 term)
_O_PCH = 0
_O_ESM = 512 * S_PER_CORE
OW = _O_ESM + 32


def _expand_host(node_par, node_par_k, t):
    """numpy mirror of the reference's expand_graph_proposals."""
    Bb, Mm, _ = node_par.shape
    j = np.arange(L2)
    c = np.broadcast_to(node_par[:, :, :, None], (Bb, Mm, L2, L2)).copy()
    c[:, :, j, j] = L + t
    c[:, :, t, :] = L + t
    if t > 0:
        c[:, :, L + t, :] = node_par
    kk = np.arange(K2)
    k_sib = (kk + K) % K2
    ck = np.broadcast_to(
        node_par_k[:, :, :, None, None], (Bb, Mm, L2, K2, L2)).copy()
    ck[:, :, j[:, None], kk[None, :], j[:, None]] = np.broadcast_to(
        k_sib[None, :], (L2, K2))
    ck[:, :, t, :, :] = np.broadcast_to(kk[:, None], (K2, L2))
    ck[:, :, L + t, :, :] = np.broadcast_to(
        node_par_k[:, :, None, :], (Bb, Mm, K2, L2))
    mask = np.concatenate(
        [np.arange(L) <= max(0, t - 1),
         (np.arange(L) <= t - 1) & (np.arange(L) > 0)]).astype(np.float32)
    opc = np.broadcast_to(mask[None, :], (K2, L2)) + EPS
    opc = np.log(opc / opc.sum())
    return c, ck, opc.astype(np.float32)


_NC_CACHE = {}


def _build_nc():
    """Build the SPMD Bass module (identical program for all 8 cores).

    Raw bacc with manual semaphores (Tile's end-of-kernel drain + EVSEM
    butterfly costs ~10us this kernel doesn't need).  The cross term is
    folded into the ACT square via (ASC*n + b)^2 = 49.5 n^2 - v.n + b^2
    with per-partition bias b = -v/(2*ASC), so raw-noise PE streams
    disappear.  Engine programs:
      sync:   smalls DMA, noise b4+b0+b1 DMA, final outv store
      scalar: noise b2+b3 DMA (ACT HW-DGE ring), 5 biased squares, 2 exp
      tensor: 2 lse MMs, 2 out36 MMs, 2 pch chains (b4-half + masked36)
      gpsimd: 4 block partition-sums of the biased squares
      vector: 2 masked muls, esum, transpose, 8 stage copies
    """
    if "nc" in _NC_CACHE:
        return _NC_CACHE["nc"]
    from concourse import bacc, mybir

    f32 = mybir.dt.float32
    bf16 = mybir.dt.bfloat16
    AF = mybir.ActivationFunctionType
    nrow = 12 * S_PER_CORE

    nc = bacc.Bacc("TRN2", target_bir_lowering=False, debug=False,
                   num_devices=NCORES)
    # noise_big columns: [T1_s0 | T2_s0 | T1_s1 | T2_s1 | T36]; each block
    # (128, 512) = (i-pair e-stacked or s-stacked, k*64+j)
    noise_big = nc.dram_tensor("noise_big", [128, 5 * 512], f32,
                               kind="ExternalInput")
    smalls_a = nc.dram_tensor("smalls_a", [128, SWA], f32,
                              kind="ExternalInput")
    smalls_b = nc.dram_tensor("smalls_b", [64, SWB], f32,
                              kind="ExternalInput")
    outv = nc.dram_tensor("outv", [1, OW], f32, kind="ExternalOutput")

    with ExitStack() as ctx:
        e = ctx.enter_context
        t_sma = e(nc.sbuf_tensor("t_sma", [128, SWA], f32)).ap()
        t_smb = e(nc.sbuf_tensor("t_smb", [64, SWB], f32)).ap()
        t_noise = e(nc.sbuf_tensor("t_noise", [128, 5 * 512], f32)).ap()
        t_sq = e(nc.sbuf_tensor("t_sq", [128, 5 * 512], f32)).ap()
        t_mask = e(nc.sbuf_tensor("t_mask", [64, 512], f32)).ap()
        mk0 = e(nc.sbuf_tensor("mk0", [64, 512], f32)).ap()
        mk1 = e(nc.sbuf_tensor("mk1", [64, 512], f32)).ap()
        esc0 = e(nc.sbuf_tensor("esc0", [nrow, 512], f32)).ap()
        esc1 = e(nc.sbuf_tensor("esc1", [nrow, 512], f32)).ap()
        eacc0 = e(nc.sbuf_tensor("eacc0", [nrow, 1], f32)).ap()
        eacc1 = e(nc.sbuf_tensor("eacc1", [nrow, 1], f32)).ap()
        lse32 = e(nc.sbuf_tensor("lse32", [32, 32], f32)).ap()
        lse32T = e(nc.sbuf_tensor("lse32T", [32, 32], f32)).ap()
        stage = e(nc.sbuf_tensor("stage", [1, OW], f32)).ap()
        dumio = e(nc.sbuf_tensor("dumio", [1, 2], f32)).ap()

        pwarm = e(nc.psum_tensor("pwarm", [1, 512], f32)).ap()
        p36_0 = e(nc.psum_tensor("p36_0", [64, 512], f32)).ap()
        p36_1 = e(nc.psum_tensor("p36_1", [64, 512], f32)).ap()
        plg0 = e(nc.psum_tensor("plg0", [nrow, 512], f32)).ap()
        plg1 = e(nc.psum_tensor("plg1", [nrow, 512], f32)).ap()
        pch = [e(nc.psum_tensor(f"pch_{s}", [1, 512], f32)).ap()
               for s in range(S_PER_CORE)]

        DSA = e(nc.semaphore("DSA"))
        DSB = e(nc.semaphore("DSB"))
        DN4 = e(nc.semaphore("DN4"))
        DN01 = e(nc.semaphore("DN01"))
        NHI = e(nc.semaphore("NHI"))
        SMSK = e(nc.semaphore("SMSK"))
        DOUT = e(nc.semaphore("DOUT"))
        SPE = e(nc.semaphore("SPE"))
        SACT = e(nc.semaphore("SACT"))
        SDVE = e(nc.semaphore("SDVE"))
        SMK = e(nc.semaphore("SMK"))

        t_vst = t_sma[:, _C_VST:_C_VST + 7]
        t_v36 = [t_sma[:, _C_V36 + 64 * s:_C_V36 + 64 * (s + 1)]
                 for s in range(S_PER_CORE)]
        t_emb = t_smb[:, _C_EMB:_C_EMB + 512].bitcast(bf16)
        t_gsel = t_smb[:, _C_GSEL:_C_GSEL + 6 * S_PER_CORE].bitcast(bf16)

        def nslc(blk):
            return t_noise[:, 512 * blk:512 * (blk + 1)]

        def sqslc(blk):
            return t_sq[:, 512 * blk:512 * (blk + 1)]

        onesA = t_vst[:, 4:5]
        onesB = t_vst[:, 5:6]
        ones128 = t_vst[:, 6:7]
        ones64 = t_vst[0:64, 6:7]
        wlhs = nc.const_aps.tensor(1.0, (128, 1))
        wrhs = nc.const_aps.tensor(1.0, (128, 512))

        with nc.Block() as block:

            @block.sync
            def _(sync):
                sync.dma_start(t_noise[:, 0:1024],
                               noise_big[:, 0:1024]).then_inc(DN01, 16)
                sync.wait_ge(SDVE, 5)
                sync.wait_ge(SACT, 8)
                sync.dma_start(outv[:], stage[:]).then_inc(DOUT, 16)
                sync.wait_ge(DOUT, 16)

            @block.scalar
            def _(scalar):
                scalar.dma_start(t_noise[:, 2048:2560],
                                 noise_big[:, 2048:2560]).then_inc(DN4, 16)
                scalar.dma_start(t_noise[:, 1024:2048],
                                 noise_big[:, 1024:2048]).then_inc(NHI, 16)
                # dummy first activation: the ACT table load is emitted
                # right before it, overlapping the DMA wait
                scalar.activation(dumio[:], nc.const_aps.tensor(0.0, (1, 2)),
                                  AF.Square)
                scalar.wait_ge(DN4, 16)
                scalar.activation(sqslc(4), nslc(4), AF.Square,
                                  scale=ASC).then_inc(SACT)       # 1
                scalar.wait_ge(DSA, 16)
                scalar.wait_ge(DN01, 16)
                scalar.activation(sqslc(0), nslc(0), AF.Square,
                                  bias=t_vst[:, 0:1],
                                  scale=ASC).then_inc(SACT)       # 2
                scalar.activation(sqslc(1), nslc(1), AF.Square,
                                  bias=t_vst[:, 1:2],
                                  scale=ASC).then_inc(SACT)       # 3
                scalar.wait_ge(NHI, 16)
                scalar.activation(sqslc(2), nslc(2), AF.Square,
                                  bias=t_vst[:, 2:3],
                                  scale=ASC).then_inc(SACT)       # 4
                scalar.activation(sqslc(3), nslc(3), AF.Square,
                                  bias=t_vst[:, 3:4],
                                  scale=ASC).then_inc(SACT)       # 5
                scalar.wait_ge(SPE, 1)
                scalar.activation(esc0[:], plg0[:], AF.Exp,
                                  accum_out=eacc0[:]).then_inc(SACT)  # 6
                scalar.activation(esc1[:], plg1[:], AF.Exp,
                                  accum_out=eacc1[:]).then_inc(SACT)  # 7
                scalar.wait_ge(SPE, 3)
                scalar.copy(stage[:, _O_PCH:_O_PCH + 512],
                            pch[0][:]).then_inc(SACT)             # 8

            @block.tensor
            def _(tensor):
                # warm the PE p-state while DMAs land
                for _i in range(3):
                    tensor.matmul(pwarm[:], wlhs, wrhs,
                                  start=True, stop=True)
                tensor.wait_ge(DSB, 16)
                tensor.matmul(plg0[:], t_gsel, t_emb[:, 0:512],
                              start=True, stop=True)
                tensor.matmul(plg1[:], t_gsel, t_emb[:, 512:1024],
                              start=True, stop=True).then_inc(SPE)    # 1
                tensor.wait_ge(DSA, 16)
                tensor.wait_ge(DN4, 16)
                tensor.matmul(p36_0[:], t_v36[0], nslc(4),
                              start=True, stop=True)
                tensor.matmul(p36_1[:], t_v36[1], nslc(4),
                              start=True, stop=True).then_inc(SPE)    # 2
                tensor.wait_ge(SACT, 3)
                tensor.wait_ge(SMK, 1)
                tensor.matmul(pch[0][:], ones128, sqslc(0),
                              start=True, stop=False)
                tensor.matmul(pch[0][:], ones128, sqslc(1),
                              start=False, stop=False)
                tensor.matmul(pch[0][:], onesA, sqslc(4),
                              start=False, stop=False)
                tensor.matmul(pch[0][:], ones64, mk0[:],
                              start=False, stop=True).then_inc(SPE)   # 3
                tensor.wait_ge(SACT, 5)
                tensor.wait_ge(SMK, 2)
                tensor.matmul(pch[1][:], ones128, sqslc(2),
                              start=True, stop=False)
                tensor.matmul(pch[1][:], ones128, sqslc(3),
                              start=False, stop=False)
                tensor.matmul(pch[1][:], onesB, sqslc(4),
                              start=False, stop=False)
                tensor.matmul(pch[1][:], ones64, mk1[:],
                              start=False, stop=True).then_inc(SPE)   # 4

            @block.gpsimd
            def _(gpsimd):
                gpsimd.dma_start(t_sma[:], smalls_a[:]).then_inc(DSA, 16)
                gpsimd.dma_start(t_smb[:], smalls_b[:]).then_inc(DSB, 16)
                gpsimd.memset(t_mask[:], 1.0).then_inc(SMSK)      # 1
                gpsimd.wait_ge(SMSK, 1)
                # keep 1.0 where (col % 64) == row, else 0
                gpsimd.affine_select(
                    out=t_mask[:], in_=t_mask[:],
                    compare_op=mybir.AluOpType.is_equal, fill=0.0,
                    base=0, pattern=[[0, 8], [1, 64]],
                    channel_multiplier=-1).then_inc(SMSK)         # 2

            @block.vector
            def _(vector):
                vector.memset(lse32[:], 0.0).then_inc(SDVE)       # 1
                vector.wait_ge(SMSK, 2)
                vector.wait_ge(SPE, 2)
                vector.tensor_mul(mk0[:], p36_0[:], t_mask[:]).then_inc(SMK)
                vector.tensor_mul(mk1[:], p36_1[:], t_mask[:]).then_inc(SMK)
                vector.wait_ge(SACT, 7)
                vector.wait_ge(SDVE, 1)
                vector.tensor_add(lse32[0:nrow, 0:1], eacc0[:],
                                  eacc1[:]).then_inc(SDVE)        # 2
                vector.wait_ge(SDVE, 2)
                vector.transpose(lse32T[:], lse32[:]).then_inc(SDVE)  # 3
                vector.wait_ge(SDVE, 3)
                vector.tensor_copy(stage[:, _O_ESM:_O_ESM + 32],
                                   lse32T[0:1, :]).then_inc(SDVE)     # 4
                vector.wait_ge(SPE, 4)
                vector.tensor_copy(stage[:, _O_PCH + 512:_O_PCH + 1024],
                                   pch[1][:]).then_inc(SDVE)          # 5

    nc.compile()
    _NC_CACHE["nc"] = nc
    return nc


def _prep_inputs(tok, lp_graph, node_ie, node_par, node_par_k, emb, w_k,
                 noise, t, T):
    """Host prep: per-core in_maps + per-(b,m) aux for assembly."""
    import ml_dtypes
    G_all = np.einsum("bmpe,qef->bmpqf", node_ie, w_k).astype(np.float32)
    embT16 = np.ascontiguousarray(emb.T).astype(ml_dtypes.bfloat16)
    jj = np.arange(512) % 64
    maskd = -(np.arange(64)[:, None] == jj[None, :]).astype(np.float32)

    in_maps = []
    aux = {}
    for core in range(NCORES):
        noise_big = np.empty((128, 5 * 512), np.float32)
        smalls_a = np.zeros((128, SWA), np.float32)
        smalls_b = np.zeros((64, SWB), np.float32)
        smalls_a[0:64, 4] = 1.0
        smalls_a[64:128, 5] = 1.0
        smalls_a[:, 6] = 1.0
        smalls_b[:, _C_EMB:_C_EMB + 512] = embT16.view(np.float32)
        gsel16 = np.zeros((64, 12 * S_PER_CORE), ml_dtypes.bfloat16)
        for s in range(S_PER_CORE):
            bm = S_PER_CORE * core + s
            b, m = bm // M, bm % M
            nie = node_ie[b, m]
            G = G_all[b, m]
            npar, nprk = node_par[b, m], node_par_k[b, m]
            # (5, 8, 64, 64) -> (5, 64e, k*64+j)
            nT = np.transpose(noise[b, m, T:T + t + 1],
                              (0, 3, 1, 2)).reshape(5, 64, 512)
            noise_big[0:64, 1024 * s:1024 * s + 512] = nT[0]
            noise_big[64:128, 1024 * s:1024 * s + 512] = nT[1]
            noise_big[0:64, 1024 * s + 512:1024 * s + 1024] = nT[2]
            noise_big[64:128, 1024 * s + 512:1024 * s + 1024] = nT[3]
            noise_big[64 * s:64 * s + 64, 2048:2560] = nT[4]
            v = np.zeros((4, E), np.float32)
            v[0] = nie[T]
            for ii in range(1, 4):
                i = T + ii
                v[ii] = nie[i] - G[npar[i], nprk[i]]
            bias = -v / (2.0 * ASC)
            smalls_a[0:64, 2 * s] = bias[0]
            smalls_a[64:128, 2 * s] = bias[1]
            smalls_a[0:64, 2 * s + 1] = bias[2]
            smalls_a[64:128, 2 * s + 1] = bias[3]
            V36 = nie[L + t][None, :] - G[npar, nprk]     # (64 j, 64 e)
            smalls_a[64 * s:64 * s + 64,
                     _C_V36 + 64 * s:_C_V36 + 64 * (s + 1)] = -V36.T
            Gsel = np.zeros((12, E), np.float32)
            for i in range(4):
                Gsel[i] = G[npar[i], nprk[i]]
            for q in range(K2):
                Gsel[4 + q] = G[L + t, q]
            gsel16[:, 12 * s:12 * s + 12] = Gsel.T.astype(ml_dtypes.bfloat16)
            aux[(b, m)] = dict(G=G, v=v, V36=V36, Gsel=Gsel)
        smalls_b[:, _C_GSEL:_C_GSEL + 6 * S_PER_CORE] = \
            gsel16.view(np.float32)
        in_maps.append({"noise_big": noise_big, "smalls_a": smalls_a,
                        "smalls_b": smalls_b})
    return in_maps, aux


def _run_device(in_maps, **spmd_kwargs):
    from concourse.bass_utils import run_bass_kernel_spmd
    nc = _build_nc()
    return run_bass_kernel_spmd(nc, in_maps, list(range(NCORES)),
                                **spmd_kwargs)


def kernel(tok_external, lp_graph, node_ie, node_par, node_par_k,
           emb_vocab, w_k, noise, t, max_t, _spmd_kwargs=None,
           _results=None):
    tok = np.asarray(tok_external)
    lp_graph = np.asarray(lp_graph, np.float32)
    node_ie = np.asarray(node_ie, np.float32)
    node_par = np.asarray(node_par)
    node_par_k = np.asarray(node_par_k)
    emb = np.asarray(emb_vocab, np.float32)
    w_k = np.asarray(w_k, np.float32)
    noise = np.asarray(noise, np.float32)
    t = int(t)
    T = int(max_t)
    assert t == 4 and T == 32 and noise.shape == (B, M, L2, K2, L2, E)

    c, ck, opc = _expand_host(node_par, node_par_k, t)
    in_maps, aux = _prep_inputs(tok, lp_graph, node_ie, node_par,
                                node_par_k, emb, w_k, noise, t, T)
    if _results is None:
        _results = _run_device(in_maps, **(_spmd_kwargs or {})).results

    # ---------------- host assembly ----------------
    const = -(t + 1) * E * math.log(10.0)
    kk = np.arange(K2)
    opt_logp = np.zeros((B, M, K2, L2), np.float64)
    for core in range(NCORES):
        out = np.asarray(_results[core]["outv"], np.float64).reshape(-1)
        for s in range(S_PER_CORE):
            bm = S_PER_CORE * core + s
            b, m = bm // M, bm % M
            a = aux[(b, m)]
            # combo = 49.5*SQS - CRS + CB (device chain + gpsimd block sums)
            combo = out[_O_PCH + 512 * s:_O_PCH + 512 * (s + 1)] \
                .reshape(K2, L2).copy()
            LSE = np.log(out[_O_ESM + 12 * s:_O_ESM + 12 * (s + 1)])
            G, v, V36, Gsel = a["G"], a["v"], a["V36"], a["Gsel"]
            nie = node_ie[b, m]
            v64 = v.astype(np.float64)
            combo -= (v64 * v64).sum() / (4.0 * (BETASQ2 - BETASQ) / 2.0)

            A2S = np.zeros((K2, L2), np.float64)
            A2S += sum(float(vv @ vv) for vv in v)
            A2S += (V36.astype(np.float64) ** 2).sum(axis=1)[None, :]
            for ii in range(1, 4):                 # diag fixups i=33..35
                i = T + ii
                vbase = v[ii].astype(np.float64)
                for k in range(K2):
                    vdiag = (nie[i] - G[L + t, (k + K) % K2]).astype(np.float64)
                    A2S[k, i] += vdiag @ vdiag - vbase @ vbase
                    nrow = noise[b, m, i, k, i].astype(np.float64)
                    combo[k, i] -= (vdiag - vbase) @ nrow

            lp_int = const - 0.5 * A2S + combo

            embtok = emb[tok[b, :t + 1]].astype(np.float64)    # (5, 64)
            TL = Gsel.astype(np.float64) @ embtok.T            # (12, 5)
            ext = np.zeros((K2, L2), np.float64)
            ext += sum(TL[i, i] - LSE[i] for i in range(4))
            ext += (TL[4 + kk, 4] - LSE[4 + kk])[:, None]
            for i in range(4):
                r2 = 4 + (kk + K) % K2
                ext[:, i] += (TL[r2, i] - LSE[r2]) - (TL[i, i] - LSE[i])

            opt_logp[b, m] = lp_int + ext + lp_graph[b, m] + opc

    # ---------------- top-k + outputs ----------------
    flat = opt_logp.reshape(B, M * K2 * L2).astype(np.float32)
    top_idx = np.argsort(-flat, axis=1, kind="stable")[:, :M]
    lp_joint = np.take_along_axis(flat, top_idx, axis=1)

    lp_graph_next = np.zeros((B, M), np.float32)
    node_ie_next = np.zeros((B, M, L2, E), np.float32)
    node_par_next = np.zeros((B, M, L2), node_par.dtype)
    node_par_k_next = np.zeros((B, M, L2), node_par_k.dtype)
    w64 = w_k.astype(np.float64)
    emb64 = emb.astype(np.float64)
    for b in range(B):
        embtok = emb[tok[b, :t + 1]].astype(np.float64)
        for q in range(M):
            idx = int(top_idx[b, q])
            m, k, j = idx // (K2 * L2), (idx // L2) % K2, idx % L2
            lp_graph_next[b, q] = lp_graph[b, m] + opc[k, j]
            node_par_next[b, q] = c[b, m, :, j]
            node_par_k_next[b, q] = ck[b, m, :, k, j]

            nie = node_ie[b, m].astype(np.float64)
            g = np.zeros((L2, E), np.float64)
            for i in range(T, T + t + 1):          # internal rows 32..36
                Pi = int(c[b, m, i, j])
                Qi = int(ck[b, m, i, k, j])
                w = 0.0 if i == T else nie[Pi] @ w64[Qi]
                r = nie[i] + noise[b, m, i, k, j].astype(np.float64) - w
                g[i] += BETASQ * r
                if i != T:
                    g[Pi] += (-BETASQ * r) @ w64[Qi].T
            for i in range(t + 1):                 # external rows 0..4
                Pi = int(c[b, m, i, j])
                Qi = int(ck[b, m, i, k, j])
                grow = nie[Pi] @ w64[Qi]
                logits = grow @ emb64.T
                sm = np.exp(logits - logits.max())
                sm /= sm.sum()
                dwke = sm @ emb64
                dwke -= embtok[i]
                g[Pi] += dwke @ w64[Qi].T
            node_ie_next[b, q] = (nie - INNER_LR * g).astype(np.float32)

    return (lp_joint, lp_graph_next, node_ie_next, node_par_next,
            node_par_k_next)


# revision 25
# speedup vs baseline: 1.1816x; 1.1816x over previous
"""Trainium2 Bass kernel for nn_DPT_52845277610695 (topk_masking).

Self-contained: accepts FULL unsharded inputs, shards (b,m) pairs across
8 NeuronCores (2 pairs per core), runs one SPMD Bass kernel, and
assembles the final outputs on host.

Math: the reference's inner value_and_grad only touches `noise` at node
rows i in [T, T+t] = [32, 36], and every candidate score decomposes as
    opt_logp[b,m,k,j] = const + lp_graph[b,m] + opc[k,j]
                        - 0.5*A2S - CRS + 49.5*SQS + lp_external
where CRS[k,j] = sum_i v_i(k,j) . noise_row and SQS[k,j] = sum_i |noise_row|^2
are the only O(noise) quantities (computed on device via PE matvecs over
transposed noise tiles + ACT squares), and lp_external needs only a
12-row logsumexp over the vocab per (b,m) (computed on device).
The gradient is nonzero for at most ~14 rows per selected candidate and
is assembled on host for the 16 selected candidates only.

Device I/O is packed into single large-descriptor DMAs (HW-DGE
descriptor generation is the wall-clock bottleneck otherwise).
"""
import math
from contextlib import ExitStack

import numpy as np

PI = 3.1415926
EPS = 1e-8
B, M, L, K, E, V = 2, 8, 32, 4, 64, 1024
L2, K2 = 2 * L, 2 * K
BETASQ = 1.0
BETASQ2 = 100.0
INNER_LR = 1e-3
NCORES = 8
S_PER_CORE = (B * M) // NCORES  # 2

ASC = math.sqrt((BETASQ2 - BETASQ) / 2.0)   # activation square scale

# ---- pkA layout (f32 columns of a (128, PKA) tile), sync-ring DMA ----
_C_VST = 0            # (128, 7): cols 0..3 = square-bias cols -v/(2*ASC)
#                       col 4 = onesA (top half), 5 = onesB (bottom), 6 = ones
_C_V36 = 7            # (128, 64) per s: zero-padded lhsT (NEGATED V36T)
_C_EMB = 7 + 64 * S_PER_CORE            # rows 0:64: (64,512) f32 = bf16 embT
_C_GSEL = _C_EMB + 512                  # rows 0:64: (64, 6S) f32
_C_B4 = _C_GSEL + 6 * S_PER_CORE       # (128, 512) noise block 4
PKA = _C_B4 + 512
# pkB: (128, 2048) noise blocks 0..3, scalar-ring DMA
# ---- packed output layout: one row (1, OW) ----
# pch: per-s PE chain = 49.5*SQS - CRS + CB (all blocks + masked 36 term)
_O_PCH = 0
_O_ESM = 512 * S_PER_CORE
OW = _O_ESM + 32


def _expand_host(node_par, node_par_k, t):
    """numpy mirror of the reference's expand_graph_proposals."""
    Bb, Mm, _ = node_par.shape
    j = np.arange(L2)
    c = np.broadcast_to(node_par[:, :, :, None], (Bb, Mm, L2, L2)).copy()
    c[:, :, j, j] = L + t
    c[:, :, t, :] = L + t
    if t > 0:
        c[:, :, L + t, :] = node_par
    kk = np.arange(K2)
    k_sib = (kk + K) % K2
    ck = np.broadcast_to(
        node_par_k[:, :, :, None, None], (Bb, Mm, L2, K2, L2)).copy()
    ck[:, :, j[:, None], kk[None, :], j[:, None]] = np.broadcast_to(
        k_sib[None, :], (L2, K2))
    ck[:, :, t, :, :] = np.broadcast_to(kk[:, None], (K2, L2))
    ck[:, :, L + t, :, :] = np.broadcast_to(
        node_par_k[:, :, None, :], (Bb, Mm, K2, L2))
    mask = np.concatenate(
        [np.arange(L) <= max(0, t - 1),
         (np.arange(L) <= t - 1) & (np.arange(L) > 0)]).astype(np.float32)
    opc = np.broadcast_to(mask[None, :], (K2, L2)) + EPS
    opc = np.log(opc / opc.sum())
    return c, ck, opc.astype(np.float32)


_NC_CACHE = {}


def _build_nc():
    """Build the SPMD Bass module (identical program for all 8 cores).

    Raw bacc with manual semaphores (Tile's end-of-kernel drain + EVSEM
    butterfly costs ~10us this kernel doesn't need).  The cross term is
    folded into the ACT square via (ASC*n + b)^2 = 49.5 n^2 - v.n + b^2
    with per-partition bias b = -v/(2*ASC), so raw-noise PE streams
    disappear.  Engine programs:
      sync:   smalls DMA, noise b4+b0+b1 DMA, final outv store
      scalar: noise b2+b3 DMA (ACT HW-DGE ring), 5 biased squares, 2 exp
      tensor: 2 lse MMs, 2 out36 MMs, 2 pch chains (b4-half + masked36)
      gpsimd: 4 block partition-sums of the biased squares
      vector: 2 masked muls, esum, transpose, 8 stage copies
    """
    if "nc" in _NC_CACHE:
        return _NC_CACHE["nc"]
    from concourse import bacc, mybir

    f32 = mybir.dt.float32
    bf16 = mybir.dt.bfloat16
    AF = mybir.ActivationFunctionType
    nrow = 12 * S_PER_CORE

    nc = bacc.Bacc("TRN2", target_bir_lowering=False, debug=False,
                   num_devices=NCORES)
    # noise_big columns: [T1_s0 | T2_s0 | T1_s1 | T2_s1 | T36]; each block
    # (128, 512) = (i-pair e-stacked or s-stacked, k*64+j)
    pka_d = nc.dram_tensor("pka", [128, PKA], f32, kind="ExternalInput")
    pkb_d = nc.dram_tensor("pkb", [128, 4 * 512], f32, kind="ExternalInput")
    outv = nc.dram_tensor("outv", [1, OW], f32, kind="ExternalOutput")

    with ExitStack() as ctx:
        e = ctx.enter_context
        t_pka = e(nc.sbuf_tensor("t_pka", [128, PKA], f32)).ap()
        t_pkb = e(nc.sbuf_tensor("t_pkb", [128, 4 * 512], f32)).ap()
        t_sq = e(nc.sbuf_tensor("t_sq", [128, 5 * 512], f32)).ap()
        t_mask = e(nc.sbuf_tensor("t_mask", [64, 512], f32)).ap()
        mk0 = e(nc.sbuf_tensor("mk0", [64, 512], f32)).ap()
        mk1 = e(nc.sbuf_tensor("mk1", [64, 512], f32)).ap()
        esc0 = e(nc.sbuf_tensor("esc0", [nrow, 512], f32)).ap()
        esc1 = e(nc.sbuf_tensor("esc1", [nrow, 512], f32)).ap()
        eacc0 = e(nc.sbuf_tensor("eacc0", [nrow, 1], f32)).ap()
        eacc1 = e(nc.sbuf_tensor("eacc1", [nrow, 1], f32)).ap()
        lse32 = e(nc.sbuf_tensor("lse32", [32, 32], f32)).ap()
        lse32T = e(nc.sbuf_tensor("lse32T", [32, 32], f32)).ap()
        stage = e(nc.sbuf_tensor("stage", [1, OW], f32)).ap()
        dumio = e(nc.sbuf_tensor("dumio", [1, 2], f32)).ap()

        pwarm = e(nc.psum_tensor("pwarm", [1, 512], f32)).ap()
        p36_0 = e(nc.psum_tensor("p36_0", [64, 512], f32)).ap()
        p36_1 = e(nc.psum_tensor("p36_1", [64, 512], f32)).ap()
        plg0 = e(nc.psum_tensor("plg0", [nrow, 512], f32)).ap()
        plg1 = e(nc.psum_tensor("plg1", [nrow, 512], f32)).ap()
        pch = [e(nc.psum_tensor(f"pch_{s}", [1, 512], f32)).ap()
               for s in range(S_PER_CORE)]

        DA = e(nc.semaphore("DA"))
        DB = e(nc.semaphore("DB"))
        SMSK = e(nc.semaphore("SMSK"))
        DOUT = e(nc.semaphore("DOUT"))
        SPE = e(nc.semaphore("SPE"))
        SACT = e(nc.semaphore("SACT"))
        SDVE = e(nc.semaphore("SDVE"))
        SMK = e(nc.semaphore("SMK"))

        t_vst = t_pka[:, _C_VST:_C_VST + 7]
        t_v36 = [t_pka[:, _C_V36 + 64 * s:_C_V36 + 64 * (s + 1)]
                 for s in range(S_PER_CORE)]
        t_emb = t_pka[0:64, _C_EMB:_C_EMB + 512].bitcast(bf16)
        t_gsel = t_pka[0:64, _C_GSEL:_C_GSEL + 6 * S_PER_CORE].bitcast(bf16)
        t_b4 = t_pka[:, _C_B4:_C_B4 + 512]

        def nslc(blk):
            return t_pkb[:, 512 * blk:512 * (blk + 1)]

        def sqslc(blk):
            return t_sq[:, 512 * blk:512 * (blk + 1)]

        onesA = t_vst[:, 4:5]
        onesB = t_vst[:, 5:6]
        ones128 = t_vst[:, 6:7]
        ones64 = t_vst[0:64, 6:7]
        wlhs = nc.const_aps.tensor(1.0, (128, 1))
        wrhs = nc.const_aps.tensor(1.0, (128, 512))

        with nc.Block() as block:

            @block.sync
            def _(sync):
                sync.dma_start(t_pka[:], pka_d[:]).then_inc(DA, 16)
                sync.wait_ge(SDVE, 4)
                sync.wait_ge(SACT, 9)
                sync.dma_start(outv[:], stage[:]).then_inc(DOUT, 16)
                sync.wait_ge(DOUT, 16)

            @block.scalar
            def _(scalar):
                scalar.dma_start(t_pkb[:], pkb_d[:]).then_inc(DB, 16)
                # dummy first activation: the ACT table load is emitted
                # right before it, overlapping the DMA wait
                scalar.activation(dumio[:], nc.const_aps.tensor(0.0, (1, 2)),
                                  AF.Square)
                scalar.wait_ge(DA, 16)
                scalar.activation(sqslc(4), t_b4, AF.Square,
                                  scale=ASC).then_inc(SACT)       # 1
                scalar.wait_ge(DB, 16)
                scalar.activation(sqslc(0), nslc(0), AF.Square,
                                  bias=t_vst[:, 0:1],
                                  scale=ASC).then_inc(SACT)       # 2
                scalar.activation(sqslc(1), nslc(1), AF.Square,
                                  bias=t_vst[:, 1:2],
                                  scale=ASC).then_inc(SACT)       # 3
                scalar.activation(sqslc(2), nslc(2), AF.Square,
                                  bias=t_vst[:, 2:3],
                                  scale=ASC).then_inc(SACT)       # 4
                scalar.activation(sqslc(3), nslc(3), AF.Square,
                                  bias=t_vst[:, 3:4],
                                  scale=ASC).then_inc(SACT)       # 5
                scalar.wait_ge(SPE, 1)
                scalar.activation(esc0[:], plg0[:], AF.Exp,
                                  accum_out=eacc0[:]).then_inc(SACT)  # 6
                scalar.activation(esc1[:], plg1[:], AF.Exp,
                                  accum_out=eacc1[:]).then_inc(SACT)  # 7
                scalar.wait_ge(SPE, 3)
                scalar.copy(stage[:, _O_PCH:_O_PCH + 512],
                            pch[0][:]).then_inc(SACT)             # 8
                scalar.wait_ge(SDVE, 3)
                scalar.copy(stage[:, _O_ESM:_O_ESM + 32],
                            lse32T[0:1, :]).then_inc(SACT)        # 9

            @block.tensor
            def _(tensor):
                # warm the PE p-state while DMAs land
                for _i in range(3):
                    tensor.matmul(pwarm[:], wlhs, wrhs,
                                  start=True, stop=True)
                tensor.wait_ge(DA, 16)
                tensor.matmul(plg0[:], t_gsel, t_emb[:, 0:512],
                              start=True, stop=True)
                tensor.matmul(plg1[:], t_gsel, t_emb[:, 512:1024],
                              start=True, stop=True).then_inc(SPE)    # 1
                tensor.matmul(p36_0[:], t_v36[0], t_b4,
                              start=True, stop=True)
                tensor.matmul(p36_1[:], t_v36[1], t_b4,
                              start=True, stop=True).then_inc(SPE)    # 2
                tensor.wait_ge(SACT, 3)
                tensor.matmul(pch[0][:], ones128, sqslc(0),
                              start=True, stop=False)
                tensor.matmul(pch[0][:], ones128, sqslc(1),
                              start=False, stop=False)
                tensor.matmul(pch[0][:], onesA, sqslc(4),
                              start=False, stop=False)
                tensor.wait_ge(SMK, 1)
                tensor.matmul(pch[0][:], ones64, mk0[:],
                              start=False, stop=True).then_inc(SPE)   # 3
                tensor.wait_ge(SACT, 5)
                tensor.matmul(pch[1][:], ones128, sqslc(2),
                              start=True, stop=False)
                tensor.matmul(pch[1][:], ones128, sqslc(3),
                              start=False, stop=False)
                tensor.matmul(pch[1][:], onesB, sqslc(4),
                              start=False, stop=False)
                tensor.wait_ge(SMK, 2)
                tensor.matmul(pch[1][:], ones64, mk1[:],
                              start=False, stop=True).then_inc(SPE)   # 4

            @block.gpsimd
            def _(gpsimd):
                gpsimd.memset(t_mask[:], 1.0).then_inc(SMSK)      # 1
                gpsimd.wait_ge(SMSK, 1)
                # keep 1.0 where (col % 64) == row, else 0
                gpsimd.affine_select(
                    out=t_mask[:], in_=t_mask[:],
                    compare_op=mybir.AluOpType.is_equal, fill=0.0,
                    base=0, pattern=[[0, 8], [1, 64]],
                    channel_multiplier=-1).then_inc(SMSK)         # 2

            @block.vector
            def _(vector):
                vector.memset(lse32[:], 0.0).then_inc(SDVE)       # 1
                vector.wait_ge(SMSK, 2)
                vector.wait_ge(SPE, 2)
                vector.tensor_mul(mk0[:], p36_0[:], t_mask[:]).then_inc(SMK)
                vector.tensor_mul(mk1[:], p36_1[:], t_mask[:]).then_inc(SMK)
                vector.wait_ge(SACT, 7)
                vector.wait_ge(SDVE, 1)
                vector.tensor_add(lse32[0:nrow, 0:1], eacc0[:],
                                  eacc1[:]).then_inc(SDVE)        # 2
                vector.wait_ge(SDVE, 2)
                vector.transpose(lse32T[:], lse32[:]).then_inc(SDVE)  # 3
                vector.wait_ge(SPE, 4)
                vector.tensor_copy(stage[:, _O_PCH + 512:_O_PCH + 1024],
                                   pch[1][:]).then_inc(SDVE)          # 4

    nc.compile()
    _NC_CACHE["nc"] = nc
    return nc


def _prep_inputs(tok, lp_graph, node_ie, node_par, node_par_k, emb, w_k,
                 noise, t, T):
    """Host prep: per-core in_maps + per-(b,m) aux for assembly."""
    import ml_dtypes
    G_all = np.einsum("bmpe,qef->bmpqf", node_ie, w_k).astype(np.float32)
    embT16 = np.ascontiguousarray(emb.T).astype(ml_dtypes.bfloat16)
    jj = np.arange(512) % 64
    maskd = -(np.arange(64)[:, None] == jj[None, :]).astype(np.float32)

    in_maps = []
    aux = {}
    for core in range(NCORES):
        pka = np.zeros((128, PKA), np.float32)
        pkb = np.empty((128, 4 * 512), np.float32)
        pka[0:64, 4] = 1.0
        pka[64:128, 5] = 1.0
        pka[:, 6] = 1.0
        pka[0:64, _C_EMB:_C_EMB + 512] = embT16.view(np.float32)
        gsel16 = np.zeros((64, 12 * S_PER_CORE), ml_dtypes.bfloat16)
        for s in range(S_PER_CORE):
            bm = S_PER_CORE * core + s
            b, m = bm // M, bm % M
            nie = node_ie[b, m]
            G = G_all[b, m]
            npar, nprk = node_par[b, m], node_par_k[b, m]
            # (5, 8, 64, 64) -> (5, 64e, k*64+j)
            nT = np.transpose(noise[b, m, T:T + t + 1],
                              (0, 3, 1, 2)).reshape(5, 64, 512)
            pkb[0:64, 1024 * s:1024 * s + 512] = nT[0]
            pkb[64:128, 1024 * s:1024 * s + 512] = nT[1]
            pkb[0:64, 1024 * s + 512:1024 * s + 1024] = nT[2]
            pkb[64:128, 1024 * s + 512:1024 * s + 1024] = nT[3]
            pka[64 * s:64 * s + 64, _C_B4:_C_B4 + 512] = nT[4]
            v = np.zeros((4, E), np.float32)
            v[0] = nie[T]
            for ii in range(1, 4):
                i = T + ii
                v[ii] = nie[i] - G[npar[i], nprk[i]]
            bias = -v / (2.0 * ASC)
            pka[0:64, 2 * s] = bias[0]
            pka[64:128, 2 * s] = bias[1]
            pka[0:64, 2 * s + 1] = bias[2]
            pka[64:128, 2 * s + 1] = bias[3]
            V36 = nie[L + t][None, :] - G[npar, nprk]     # (64 j, 64 e)
            pka[64 * s:64 * s + 64,
                _C_V36 + 64 * s:_C_V36 + 64 * (s + 1)] = -V36.T
            Gsel = np.zeros((12, E), np.float32)
            for i in range(4):
                Gsel[i] = G[npar[i], nprk[i]]
            for q in range(K2):
                Gsel[4 + q] = G[L + t, q]
            gsel16[:, 12 * s:12 * s + 12] = Gsel.T.astype(ml_dtypes.bfloat16)
            aux[(b, m)] = dict(G=G, v=v, V36=V36, Gsel=Gsel)
        pka[0:64, _C_GSEL:_C_GSEL + 6 * S_PER_CORE] = \
            gsel16.view(np.float32)
        in_maps.append({"pka": pka, "pkb": pkb})
    return in_maps, aux


def _run_device(in_maps, **spmd_kwargs):
    from concourse.bass_utils import run_bass_kernel_spmd
    nc = _build_nc()
    return run_bass_kernel_spmd(nc, in_maps, list(range(NCORES)),
                                **spmd_kwargs)


def kernel(tok_external, lp_graph, node_ie, node_par, node_par_k,
           emb_vocab, w_k, noise, t, max_t, _spmd_kwargs=None,
           _results=None):
    tok = np.asarray(tok_external)
    lp_graph = np.asarray(lp_graph, np.float32)
    node_ie = np.asarray(node_ie, np.float32)
    node_par = np.asarray(node_par)
    node_par_k = np.asarray(node_par_k)
    emb = np.asarray(emb_vocab, np.float32)
    w_k = np.asarray(w_k, np.float32)
    noise = np.asarray(noise, np.float32)
    t = int(t)
    T = int(max_t)
    assert t == 4 and T == 32 and noise.shape == (B, M, L2, K2, L2, E)

    c, ck, opc = _expand_host(node_par, node_par_k, t)
    in_maps, aux = _prep_inputs(tok, lp_graph, node_ie, node_par,
                                node_par_k, emb, w_k, noise, t, T)
    if _results is None:
        _results = _run_device(in_maps, **(_spmd_kwargs or {})).results

    # ---------------- host assembly ----------------
    const = -(t + 1) * E * math.log(10.0)
    kk = np.arange(K2)
    opt_logp = np.zeros((B, M, K2, L2), np.float64)
    for core in range(NCORES):
        out = np.asarray(_results[core]["outv"], np.float64).reshape(-1)
        for s in range(S_PER_CORE):
            bm = S_PER_CORE * core + s
            b, m = bm // M, bm % M
            a = aux[(b, m)]
            # combo = 49.5*SQS - CRS + CB (device chain + gpsimd block sums)
            combo = out[_O_PCH + 512 * s:_O_PCH + 512 * (s + 1)] \
                .reshape(K2, L2).copy()
            LSE = np.log(out[_O_ESM + 12 * s:_O_ESM + 12 * (s + 1)])
            G, v, V36, Gsel = a["G"], a["v"], a["V36"], a["Gsel"]
            nie = node_ie[b, m]
            v64 = v.astype(np.float64)
            combo -= (v64 * v64).sum() / (4.0 * (BETASQ2 - BETASQ) / 2.0)

            A2S = np.zeros((K2, L2), np.float64)
            A2S += sum(float(vv @ vv) for vv in v)
            A2S += (V36.astype(np.float64) ** 2).sum(axis=1)[None, :]
            for ii in range(1, 4):                 # diag fixups i=33..35
                i = T + ii
                vbase = v[ii].astype(np.float64)
                for k in range(K2):
                    vdiag = (nie[i] - G[L + t, (k + K) % K2]).astype(np.float64)
                    A2S[k, i] += vdiag @ vdiag - vbase @ vbase
                    nrow = noise[b, m, i, k, i].astype(np.float64)
                    combo[k, i] -= (vdiag - vbase) @ nrow

            lp_int = const - 0.5 * A2S + combo

            embtok = emb[tok[b, :t + 1]].astype(np.float64)    # (5, 64)
            TL = Gsel.astype(np.float64) @ embtok.T            # (12, 5)
            ext = np.zeros((K2, L2), np.float64)
            ext += sum(TL[i, i] - LSE[i] for i in range(4))
            ext += (TL[4 + kk, 4] - LSE[4 + kk])[:, None]
            for i in range(4):
                r2 = 4 + (kk + K) % K2
                ext[:, i] += (TL[r2, i] - LSE[r2]) - (TL[i, i] - LSE[i])

            opt_logp[b, m] = lp_int + ext + lp_graph[b, m] + opc

    # ---------------- top-k + outputs ----------------
    flat = opt_logp.reshape(B, M * K2 * L2).astype(np.float32)
    top_idx = np.argsort(-flat, axis=1, kind="stable")[:, :M]
    lp_joint = np.take_along_axis(flat, top_idx, axis=1)

    lp_graph_next = np.zeros((B, M), np.float32)
    node_ie_next = np.zeros((B, M, L2, E), np.float32)
    node_par_next = np.zeros((B, M, L2), node_par.dtype)
    node_par_k_next = np.zeros((B, M, L2), node_par_k.dtype)
    w64 = w_k.astype(np.float64)
    emb64 = emb.astype(np.float64)
    for b in range(B):
        embtok = emb[tok[b, :t + 1]].astype(np.float64)
        for q in range(M):
            idx = int(top_idx[b, q])
            m, k, j = idx // (K2 * L2), (idx // L2) % K2, idx % L2
            lp_graph_next[b, q] = lp_graph[b, m] + opc[k, j]
            node_par_next[b, q] = c[b, m, :, j]
            node_par_k_next[b, q] = ck[b, m, :, k, j]

            nie = node_ie[b, m].astype(np.float64)
            g = np.zeros((L2, E), np.float64)
            for i in range(T, T + t + 1):          # internal rows 32..36
                Pi = int(c[b, m, i, j])
                Qi = int(ck[b, m, i, k, j])
                w = 0.0 if i == T else nie[Pi] @ w64[Qi]
                r = nie[i] + noise[b, m, i, k, j].astype(np.float64) - w
                g[i] += BETASQ * r
                if i != T:
                    g[Pi] += (-BETASQ * r) @ w64[Qi].T
            for i in range(t + 1):                 # external rows 0..4
                Pi = int(c[b, m, i, j])
                Qi = int(ck[b, m, i, k, j])
                grow = nie[Pi] @ w64[Qi]
                logits = grow @ emb64.T
                sm = np.exp(logits - logits.max())
                sm /= sm.sum()
                dwke = sm @ emb64
                dwke -= embtok[i]
                g[Pi] += dwke @ w64[Qi].T
            node_ie_next[b, q] = (nie - INNER_LR * g).astype(np.float32)

    return (lp_joint, lp_graph_next, node_ie_next, node_par_next,
            node_par_k_next)


# revision 27
# speedup vs baseline: 1.5342x; 1.2984x over previous
"""Trainium2 Bass kernel for nn_DPT_52845277610695 (topk_masking).

Self-contained: accepts FULL unsharded inputs, shards (b,m) pairs across
8 NeuronCores (2 pairs per core), runs one SPMD Bass kernel, and
assembles the final outputs on host.

Math: the reference's inner value_and_grad only touches `noise` at node
rows i in [T, T+t] = [32, 36], and every candidate score decomposes as
    opt_logp[b,m,k,j] = const + lp_graph[b,m] + opc[k,j]
                        - 0.5*A2S - CRS + 49.5*SQS + lp_external
where CRS[k,j] = sum_i v_i(k,j) . noise_row and SQS[k,j] = sum_i |noise_row|^2
are the only O(noise) quantities (computed on device via PE matvecs over
transposed noise tiles + ACT squares), and lp_external needs only a
12-row logsumexp over the vocab per (b,m) (computed on device).
The gradient is nonzero for at most ~14 rows per selected candidate and
is assembled on host for the 16 selected candidates only.

Device I/O is packed into single large-descriptor DMAs (HW-DGE
descriptor generation is the wall-clock bottleneck otherwise).
"""
import math
from contextlib import ExitStack

import numpy as np

PI = 3.1415926
EPS = 1e-8
B, M, L, K, E, V = 2, 8, 32, 4, 64, 1024
L2, K2 = 2 * L, 2 * K
BETASQ = 1.0
BETASQ2 = 100.0
INNER_LR = 1e-3
NCORES = 8
S_PER_CORE = (B * M) // NCORES  # 2

ASC = math.sqrt((BETASQ2 - BETASQ) / 2.0)   # activation square scale

# ---- pkA layout (f32 columns of a (128, PKA) tile), sync-ring DMA ----
_C_VST = 0            # (128, 7): cols 0..3 = square-bias cols -v/(2*ASC)
#                       col 4 = onesA (top half), 5 = onesB (bottom), 6 = ones
_C_V36 = 7            # (128, 64) per s: zero-padded lhsT (NEGATED V36T)
_C_EMB = 7 + 64 * S_PER_CORE            # rows 0:64: (64,512) f32 = bf16 embT
_C_GSEL = _C_EMB + 512                  # rows 0:64: (64, 6S) f32
_C_ONES16 = _C_GSEL + 6 * S_PER_CORE   # 2 f32 cols: bf16 onesA / onesB
_C_B4 = _C_ONES16 + 2                  # (128, 512) noise block 4
PKA = _C_B4 + 512
# pkB: (128, 2048) noise blocks 0..3, scalar-ring DMA
# ---- packed output layout: one row (1, OW) ----
# pch: per-s PE chain = 49.5*SQS - CRS + CB (all blocks + masked 36 term)
_O_PCH = 0
_O_ESM = 512 * S_PER_CORE
OW = _O_ESM + 32


def _expand_host(node_par, node_par_k, t):
    """numpy mirror of the reference's expand_graph_proposals."""
    Bb, Mm, _ = node_par.shape
    j = np.arange(L2)
    c = np.broadcast_to(node_par[:, :, :, None], (Bb, Mm, L2, L2)).copy()
    c[:, :, j, j] = L + t
    c[:, :, t, :] = L + t
    if t > 0:
        c[:, :, L + t, :] = node_par
    kk = np.arange(K2)
    k_sib = (kk + K) % K2
    ck = np.broadcast_to(
        node_par_k[:, :, :, None, None], (Bb, Mm, L2, K2, L2)).copy()
    ck[:, :, j[:, None], kk[None, :], j[:, None]] = np.broadcast_to(
        k_sib[None, :], (L2, K2))
    ck[:, :, t, :, :] = np.broadcast_to(kk[:, None], (K2, L2))
    ck[:, :, L + t, :, :] = np.broadcast_to(
        node_par_k[:, :, None, :], (Bb, Mm, K2, L2))
    mask = np.concatenate(
        [np.arange(L) <= max(0, t - 1),
         (np.arange(L) <= t - 1) & (np.arange(L) > 0)]).astype(np.float32)
    opc = np.broadcast_to(mask[None, :], (K2, L2)) + EPS
    opc = np.log(opc / opc.sum())
    return c, ck, opc.astype(np.float32)


_NC_CACHE = {}


def _build_nc():
    """Build the SPMD Bass module (identical program for all 8 cores).

    Raw bacc with manual semaphores (Tile's end-of-kernel drain + EVSEM
    butterfly costs ~10us this kernel doesn't need).  The cross term is
    folded into the ACT square via (ASC*n + b)^2 = 49.5 n^2 - v.n + b^2
    with per-partition bias b = -v/(2*ASC), so raw-noise PE streams
    disappear.  Engine programs:
      sync:   smalls DMA, noise b4+b0+b1 DMA, final outv store
      scalar: noise b2+b3 DMA (ACT HW-DGE ring), 5 biased squares, 2 exp
      tensor: 2 lse MMs, 2 out36 MMs, 2 pch chains (b4-half + masked36)
      gpsimd: 4 block partition-sums of the biased squares
      vector: 2 masked muls, esum, transpose, 8 stage copies
    """
    if "nc" in _NC_CACHE:
        return _NC_CACHE["nc"]
    from concourse import bacc, mybir

    f32 = mybir.dt.float32
    bf16 = mybir.dt.bfloat16
    AF = mybir.ActivationFunctionType
    nrow = 12 * S_PER_CORE

    nc = bacc.Bacc("TRN2", target_bir_lowering=False, debug=False,
                   num_devices=NCORES)
    # noise_big columns: [T1_s0 | T2_s0 | T1_s1 | T2_s1 | T36]; each block
    # (128, 512) = (i-pair e-stacked or s-stacked, k*64+j)
    pka_d = nc.dram_tensor("pka", [128, PKA], f32, kind="ExternalInput")
    pkb_d = nc.dram_tensor("pkb", [128, 4 * 512], f32, kind="ExternalInput")
    outv = nc.dram_tensor("outv", [1, OW], f32, kind="ExternalOutput")

    with ExitStack() as ctx:
        e = ctx.enter_context
        t_pka = e(nc.sbuf_tensor("t_pka", [128, PKA], f32)).ap()
        t_pkb = e(nc.sbuf_tensor("t_pkb", [128, 4 * 512], f32)).ap()
        t_sq = e(nc.sbuf_tensor("t_sq", [128, 5 * 512], bf16)).ap()
        t_mask = e(nc.sbuf_tensor("t_mask", [64, 512], f32)).ap()
        mk0 = e(nc.sbuf_tensor("mk0", [64, 512], bf16)).ap()
        mk1 = e(nc.sbuf_tensor("mk1", [64, 512], bf16)).ap()
        esc0 = e(nc.sbuf_tensor("esc0", [nrow, 512], f32)).ap()
        esc1 = e(nc.sbuf_tensor("esc1", [nrow, 512], f32)).ap()
        eacc0 = e(nc.sbuf_tensor("eacc0", [nrow, 1], f32)).ap()
        eacc1 = e(nc.sbuf_tensor("eacc1", [nrow, 1], f32)).ap()
        lse32 = e(nc.sbuf_tensor("lse32", [32, 32], f32)).ap()
        lse32T = e(nc.sbuf_tensor("lse32T", [32, 32], f32)).ap()
        stage = e(nc.sbuf_tensor("stage", [1, OW], f32)).ap()
        dumio = e(nc.sbuf_tensor("dumio", [1, 2], f32)).ap()

        pwarm = e(nc.psum_tensor("pwarm", [1, 512], f32)).ap()
        p36_0 = e(nc.psum_tensor("p36_0", [64, 512], f32)).ap()
        p36_1 = e(nc.psum_tensor("p36_1", [64, 512], f32)).ap()
        plg0 = e(nc.psum_tensor("plg0", [nrow, 512], f32)).ap()
        plg1 = e(nc.psum_tensor("plg1", [nrow, 512], f32)).ap()
        pch = [e(nc.psum_tensor(f"pch_{s}", [1, 512], f32)).ap()
               for s in range(S_PER_CORE)]

        DA = e(nc.semaphore("DA"))
        DB0 = e(nc.semaphore("DB0"))
        DB1 = e(nc.semaphore("DB1"))
        SMSK = e(nc.semaphore("SMSK"))
        DOUT = e(nc.semaphore("DOUT"))
        SPE = e(nc.semaphore("SPE"))
        SACT = e(nc.semaphore("SACT"))
        SDVE = e(nc.semaphore("SDVE"))
        SMK = e(nc.semaphore("SMK"))

        t_vst = t_pka[:, _C_VST:_C_VST + 7]
        t_v36 = [t_pka[:, _C_V36 + 64 * s:_C_V36 + 64 * (s + 1)]
                 for s in range(S_PER_CORE)]
        t_emb = t_pka[0:64, _C_EMB:_C_EMB + 512].bitcast(bf16)
        t_gsel = t_pka[0:64, _C_GSEL:_C_GSEL + 6 * S_PER_CORE].bitcast(bf16)
        t_b4 = t_pka[:, _C_B4:_C_B4 + 512]

        def nslc(blk):
            return t_pkb[:, 512 * blk:512 * (blk + 1)]

        def sqslc(blk):
            return t_sq[:, 512 * blk:512 * (blk + 1)]

        from concourse import mybir as _mb
        onesA16 = t_pka[:, _C_ONES16:_C_ONES16 + 1].bitcast(bf16)[:, 0:1]
        onesB16 = t_pka[:, _C_ONES16 + 1:_C_ONES16 + 2].bitcast(bf16)[:, 0:1]
        ones128_16 = nc.const_aps.tensor(1.0, (128, 1), dtype=_mb.dt.bfloat16)
        ones64_16 = nc.const_aps.tensor(1.0, (64, 1), dtype=_mb.dt.bfloat16)
        wlhs = nc.const_aps.tensor(1.0, (128, 1))
        wrhs = nc.const_aps.tensor(1.0, (128, 512))

        with nc.Block() as block:

            @block.sync
            def _(sync):
                sync.dma_start(t_pka[:], pka_d[:]).then_inc(DA, 16)
                sync.wait_ge(SDVE, 4)
                sync.wait_ge(SACT, 9)
                sync.dma_start(outv[:], stage[:]).then_inc(DOUT, 16)
                sync.wait_ge(DOUT, 16)

            @block.scalar
            def _(scalar):
                scalar.dma_start(t_pkb[:, 0:1024],
                                 pkb_d[:, 0:1024]).then_inc(DB0, 16)
                scalar.dma_start(t_pkb[:, 1024:2048],
                                 pkb_d[:, 1024:2048]).then_inc(DB1, 16)
                # dummy first activation: the ACT table load is emitted
                # right before it, overlapping the DMA wait
                scalar.activation(dumio[:], nc.const_aps.tensor(0.0, (1, 2)),
                                  AF.Square)
                scalar.wait_ge(DA, 16)
                scalar.activation(sqslc(4), t_b4, AF.Square,
                                  scale=ASC).then_inc(SACT)       # 1
                scalar.wait_ge(DB0, 16)
                scalar.activation(sqslc(0), nslc(0), AF.Square,
                                  bias=t_vst[:, 0:1],
                                  scale=ASC).then_inc(SACT)       # 2
                scalar.activation(sqslc(1), nslc(1), AF.Square,
                                  bias=t_vst[:, 1:2],
                                  scale=ASC).then_inc(SACT)       # 3
                scalar.wait_ge(DB1, 16)
                scalar.activation(sqslc(2), nslc(2), AF.Square,
                                  bias=t_vst[:, 2:3],
                                  scale=ASC).then_inc(SACT)       # 4
                scalar.activation(sqslc(3), nslc(3), AF.Square,
                                  bias=t_vst[:, 3:4],
                                  scale=ASC).then_inc(SACT)       # 5
                scalar.wait_ge(SPE, 1)
                scalar.activation(esc0[:], plg0[:], AF.Exp,
                                  accum_out=eacc0[:]).then_inc(SACT)  # 6
                scalar.activation(esc1[:], plg1[:], AF.Exp,
                                  accum_out=eacc1[:]).then_inc(SACT)  # 7
                scalar.wait_ge(SPE, 3)
                scalar.copy(stage[:, _O_PCH:_O_PCH + 512],
                            pch[0][:]).then_inc(SACT)             # 8
                scalar.wait_ge(SDVE, 3)
                scalar.copy(stage[:, _O_ESM:_O_ESM + 32],
                            lse32T[0:1, :]).then_inc(SACT)        # 9

            @block.tensor
            def _(tensor):
                # warm the PE p-state while DMAs land
                for _i in range(3):
                    tensor.matmul(pwarm[:], wlhs, wrhs,
                                  start=True, stop=True)
                tensor.wait_ge(DA, 16)
                tensor.matmul(plg0[:], t_gsel, t_emb[:, 0:512],
                              start=True, stop=True)
                tensor.matmul(plg1[:], t_gsel, t_emb[:, 512:1024],
                              start=True, stop=True).then_inc(SPE)    # 1
                tensor.matmul(p36_0[:], t_v36[0], t_b4,
                              start=True, stop=True)
                tensor.matmul(p36_1[:], t_v36[1], t_b4,
                              start=True, stop=True).then_inc(SPE)    # 2
                tensor.wait_ge(SACT, 3)
                tensor.matmul(pch[0][:], ones128_16, sqslc(0),
                              start=True, stop=False)
                tensor.matmul(pch[0][:], ones128_16, sqslc(1),
                              start=False, stop=False)
                tensor.matmul(pch[0][:], onesA16, sqslc(4),
                              start=False, stop=False)
                tensor.wait_ge(SMK, 1)
                tensor.matmul(pch[0][:], ones64_16, mk0[:],
                              start=False, stop=True).then_inc(SPE)   # 3
                tensor.wait_ge(SACT, 5)
                tensor.matmul(pch[1][:], ones128_16, sqslc(2),
                              start=True, stop=False)
                tensor.matmul(pch[1][:], ones128_16, sqslc(3),
                              start=False, stop=False)
                tensor.matmul(pch[1][:], onesB16, sqslc(4),
                              start=False, stop=False)
                tensor.wait_ge(SMK, 2)
                tensor.matmul(pch[1][:], ones64_16, mk1[:],
                              start=False, stop=True).then_inc(SPE)   # 4

            @block.gpsimd
            def _(gpsimd):
                gpsimd.memset(t_mask[:], 1.0).then_inc(SMSK)      # 1
                gpsimd.wait_ge(SMSK, 1)
                # keep 1.0 where (col % 64) == row, else 0
                gpsimd.affine_select(
                    out=t_mask[:], in_=t_mask[:],
                    compare_op=mybir.AluOpType.is_equal, fill=0.0,
                    base=0, pattern=[[0, 8], [1, 64]],
                    channel_multiplier=-1).then_inc(SMSK)         # 2

            @block.vector
            def _(vector):
                vector.memset(lse32[:], 0.0).then_inc(SDVE)       # 1
                vector.wait_ge(SMSK, 2)
                vector.wait_ge(SPE, 2)
                vector.tensor_mul(mk0[:], p36_0[:], t_mask[:]).then_inc(SMK)
                vector.tensor_mul(mk1[:], p36_1[:], t_mask[:]).then_inc(SMK)
                vector.wait_ge(SACT, 7)
                vector.wait_ge(SDVE, 1)
                vector.tensor_add(lse32[0:nrow, 0:1], eacc0[:],
                                  eacc1[:]).then_inc(SDVE)        # 2
                vector.wait_ge(SDVE, 2)
                vector.transpose(lse32T[:], lse32[:]).then_inc(SDVE)  # 3
                vector.wait_ge(SPE, 4)
                vector.tensor_copy(stage[:, _O_PCH + 512:_O_PCH + 1024],
                                   pch[1][:]).then_inc(SDVE)          # 4

    nc.compile()
    _NC_CACHE["nc"] = nc
    return nc


def _prep_inputs(tok, lp_graph, node_ie, node_par, node_par_k, emb, w_k,
                 noise, t, T):
    """Host prep: per-core in_maps + per-(b,m) aux for assembly."""
    import ml_dtypes
    G_all = np.einsum("bmpe,qef->bmpqf", node_ie, w_k).astype(np.float32)
    embT16 = np.ascontiguousarray(emb.T).astype(ml_dtypes.bfloat16)
    jj = np.arange(512) % 64
    maskd = -(np.arange(64)[:, None] == jj[None, :]).astype(np.float32)

    in_maps = []
    aux = {}
    for core in range(NCORES):
        pka = np.zeros((128, PKA), np.float32)
        pkb = np.empty((128, 4 * 512), np.float32)
        pka[0:64, 4] = 1.0
        pka[64:128, 5] = 1.0
        pka[:, 6] = 1.0
        o16 = np.zeros((128, 4), ml_dtypes.bfloat16)
        o16[0:64, 0] = 1.0
        o16[64:128, 2] = 1.0
        pka[:, _C_ONES16:_C_ONES16 + 2] = o16.view(np.float32)
        pka[0:64, _C_EMB:_C_EMB + 512] = embT16.view(np.float32)
        gsel16 = np.zeros((64, 12 * S_PER_CORE), ml_dtypes.bfloat16)
        for s in range(S_PER_CORE):
            bm = S_PER_CORE * core + s
            b, m = bm // M, bm % M
            nie = node_ie[b, m]
            G = G_all[b, m]
            npar, nprk = node_par[b, m], node_par_k[b, m]
            # (5, 8, 64, 64) -> (5, 64e, k*64+j)
            nT = np.transpose(noise[b, m, T:T + t + 1],
                              (0, 3, 1, 2)).reshape(5, 64, 512)
            pkb[0:64, 1024 * s:1024 * s + 512] = nT[0]
            pkb[64:128, 1024 * s:1024 * s + 512] = nT[1]
            pkb[0:64, 1024 * s + 512:1024 * s + 1024] = nT[2]
            pkb[64:128, 1024 * s + 512:1024 * s + 1024] = nT[3]
            pka[64 * s:64 * s + 64, _C_B4:_C_B4 + 512] = nT[4]
            v = np.zeros((4, E), np.float32)
            v[0] = nie[T]
            for ii in range(1, 4):
                i = T + ii
                v[ii] = nie[i] - G[npar[i], nprk[i]]
            bias = -v / (2.0 * ASC)
            pka[0:64, 2 * s] = bias[0]
            pka[64:128, 2 * s] = bias[1]
            pka[0:64, 2 * s + 1] = bias[2]
            pka[64:128, 2 * s + 1] = bias[3]
            V36 = nie[L + t][None, :] - G[npar, nprk]     # (64 j, 64 e)
            pka[64 * s:64 * s + 64,
                _C_V36 + 64 * s:_C_V36 + 64 * (s + 1)] = -V36.T
            Gsel = np.zeros((12, E), np.float32)
            for i in range(4):
                Gsel[i] = G[npar[i], nprk[i]]
            for q in range(K2):
                Gsel[4 + q] = G[L + t, q]
            gsel16[:, 12 * s:12 * s + 12] = Gsel.T.astype(ml_dtypes.bfloat16)
            aux[(b, m)] = dict(G=G, v=v, V36=V36, Gsel=Gsel)
        pka[0:64, _C_GSEL:_C_GSEL + 6 * S_PER_CORE] = \
            gsel16.view(np.float32)
        in_maps.append({"pka": pka, "pkb": pkb})
    return in_maps, aux


def _run_device(in_maps, **spmd_kwargs):
    from concourse.bass_utils import run_bass_kernel_spmd
    nc = _build_nc()
    return run_bass_kernel_spmd(nc, in_maps, list(range(NCORES)),
                                **spmd_kwargs)


def kernel(tok_external, lp_graph, node_ie, node_par, node_par_k,
           emb_vocab, w_k, noise, t, max_t, _spmd_kwargs=None,
           _results=None):
    tok = np.asarray(tok_external)
    lp_graph = np.asarray(lp_graph, np.float32)
    node_ie = np.asarray(node_ie, np.float32)
    node_par = np.asarray(node_par)
    node_par_k = np.asarray(node_par_k)
    emb = np.asarray(emb_vocab, np.float32)
    w_k = np.asarray(w_k, np.float32)
    noise = np.asarray(noise, np.float32)
    t = int(t)
    T = int(max_t)
    assert t == 4 and T == 32 and noise.shape == (B, M, L2, K2, L2, E)

    c, ck, opc = _expand_host(node_par, node_par_k, t)
    in_maps, aux = _prep_inputs(tok, lp_graph, node_ie, node_par,
                                node_par_k, emb, w_k, noise, t, T)
    if _results is None:
        _results = _run_device(in_maps, **(_spmd_kwargs or {})).results

    # ---------------- host assembly ----------------
    const = -(t + 1) * E * math.log(10.0)
    kk = np.arange(K2)
    opt_logp = np.zeros((B, M, K2, L2), np.float64)
    for core in range(NCORES):
        out = np.asarray(_results[core]["outv"], np.float64).reshape(-1)
        for s in range(S_PER_CORE):
            bm = S_PER_CORE * core + s
            b, m = bm // M, bm % M
            a = aux[(b, m)]
            # combo = 49.5*SQS - CRS + CB (device chain + gpsimd block sums)
            combo = out[_O_PCH + 512 * s:_O_PCH + 512 * (s + 1)] \
                .reshape(K2, L2).copy()
            LSE = np.log(out[_O_ESM + 12 * s:_O_ESM + 12 * (s + 1)])
            G, v, V36, Gsel = a["G"], a["v"], a["V36"], a["Gsel"]
            nie = node_ie[b, m]
            v64 = v.astype(np.float64)
            combo -= (v64 * v64).sum() / (4.0 * (BETASQ2 - BETASQ) / 2.0)

            A2S = np.zeros((K2, L2), np.float64)
            A2S += sum(float(vv @ vv) for vv in v)
            A2S += (V36.astype(np.float64) ** 2).sum(axis=1)[None, :]
            for ii in range(1, 4):                 # diag fixups i=33..35
                i = T + ii
                vbase = v[ii].astype(np.float64)
                for k in range(K2):
                    vdiag = (nie[i] - G[L + t, (k + K) % K2]).astype(np.float64)
                    A2S[k, i] += vdiag @ vdiag - vbase @ vbase
                    nrow = noise[b, m, i, k, i].astype(np.float64)
                    combo[k, i] -= (vdiag - vbase) @ nrow

            lp_int = const - 0.5 * A2S + combo

            embtok = emb[tok[b, :t + 1]].astype(np.float64)    # (5, 64)
            TL = Gsel.astype(np.float64) @ embtok.T            # (12, 5)
            ext = np.zeros((K2, L2), np.float64)
            ext += sum(TL[i, i] - LSE[i] for i in range(4))
            ext += (TL[4 + kk, 4] - LSE[4 + kk])[:, None]
            for i in range(4):
                r2 = 4 + (kk + K) % K2
                ext[:, i] += (TL[r2, i] - LSE[r2]) - (TL[i, i] - LSE[i])

            opt_logp[b, m] = lp_int + ext + lp_graph[b, m] + opc

    # ---------------- top-k + outputs ----------------
    flat = opt_logp.reshape(B, M * K2 * L2).astype(np.float32)
    top_idx = np.argsort(-flat, axis=1, kind="stable")[:, :M]
    lp_joint = np.take_along_axis(flat, top_idx, axis=1)

    lp_graph_next = np.zeros((B, M), np.float32)
    node_ie_next = np.zeros((B, M, L2, E), np.float32)
    node_par_next = np.zeros((B, M, L2), node_par.dtype)
    node_par_k_next = np.zeros((B, M, L2), node_par_k.dtype)
    w64 = w_k.astype(np.float64)
    emb64 = emb.astype(np.float64)
    for b in range(B):
        embtok = emb[tok[b, :t + 1]].astype(np.float64)
        for q in range(M):
            idx = int(top_idx[b, q])
            m, k, j = idx // (K2 * L2), (idx // L2) % K2, idx % L2
            lp_graph_next[b, q] = lp_graph[b, m] + opc[k, j]
            node_par_next[b, q] = c[b, m, :, j]
            node_par_k_next[b, q] = ck[b, m, :, k, j]

            nie = node_ie[b, m].astype(np.float64)
            g = np.zeros((L2, E), np.float64)
            for i in range(T, T + t + 1):          # internal rows 32..36
                Pi = int(c[b, m, i, j])
                Qi = int(ck[b, m, i, k, j])
                w = 0.0 if i == T else nie[Pi] @ w64[Qi]
                r = nie[i] + noise[b, m, i, k, j].astype(np.float64) - w
                g[i] += BETASQ * r
                if i != T:
                    g[Pi] += (-BETASQ * r) @ w64[Qi].T
            for i in range(t + 1):                 # external rows 0..4
                Pi = int(c[b, m, i, j])
                Qi = int(ck[b, m, i, k, j])
                grow = nie[Pi] @ w64[Qi]
                logits = grow @ emb64.T
                sm = np.exp(logits - logits.max())
                sm /= sm.sum()
                dwke = sm @ emb64
                dwke -= embtok[i]
                g[Pi] += dwke @ w64[Qi].T
            node_ie_next[b, q] = (nie - INNER_LR * g).astype(np.float32)

    return (lp_joint, lp_graph_next, node_ie_next, node_par_next,
            node_par_k_next)
